# revision 19
# baseline (speedup 1.0000x reference)
"""Trainium2 Bass kernel for nn_Event_Encoder (GNN message passing).

out = g0 * h + g1 * prop(prop(h)),  h = MLP(x),
prop(v)[d] = dinv[d] * sum_{e: dst_e = d} dinv[src_e] * v[src_e],
dinv = rsqrt(out-degree), g0 = 0.1, g1 = 0.1 * 0.81.

Strategy (8 NeuronCores, SPMD — one program, per-core data):
  - Nodes sharded 8 x 12544 (12500 real + pad). Each core: MLP on its slice
    (bf16, weights stationary on PE), rows scaled by masked rsqrt(deg).
  - Scaled rows stored compact bf16; PAIRS of consecutive slots form one
    256B dma_gather element, so the gather table is [2 x 25088, 128] bf16,
    half-major: pair row = half*25088 + core*3136 + pair. Two AllGathers
    (one per half) build it; each fires as soon as its half of the producer
    pass is done, and gathers of phase (h, k) only need table half k.
  - Edges partitioned by destination core. Per core, edges bucketed by
    (dst block of 128, src half k, src parity). A host-side balancing
    permutation assigns dst nodes to blocks so the per-(b,k,p) edge counts
    hit near-exact multiples of 128 with one schedule shared by all cores.
  - Schedule is phase-major: (dst half h, src half k), blocks within.
    Each dst block accumulates ALL its chunks (both k phases, both
    parities) into one persistent PSUM slice; a single scalar-engine
    epilogue per block reads PSUM, scales, and DMAs out. 49 live PSUM
    slices per dst half fit in one [128, 49, 64] f32 PSUM tile.
  - Per 128-edge chunk (parity-pure): dma_gather (4 SWDGE queues,
    round-robin) fetches the pair rows, an is_equal against an iota row
    builds the one-hot dst map on DVE, and the TensorEngine accumulates
    one-hot^T @ rows[parity*64:...] into the block's PSUM slice.
  - prop1 epilogue scales by masked 1/deg into bounce2 (bf16); prop2
    epilogue combines with the g0-prescaled fp32 h kept in SBUF. Host
    undoes the permutation.
"""
import numpy as np
import ml_dtypes

N = 100000
D = 64
HID = 256
CORES = 8
SLICE = 12500
PSLICE = 12544          # 98 * 128
NT = 98                 # dst blocks per core
HB = 49                 # dst blocks per half
HALF = 6272             # nodes per half (49 blocks)
NPAIR = HALF // 2       # 3136 pairs per core per half
HROWS = NPAIR * CORES   # 25088 pair rows per half
DP = 128                # pair element: 2 nodes x 64 feats bf16 = 256B
P = 128
SEG_CHUNKS = 64
G0 = 0.1
G1 = 0.1 * (1.0 - 0.1) ** 2
PAD_DL = 200.0
CLS = 4                 # (src half k, src parity p) classes: cls = k*2 + p
NQ = 4                  # SWDGE gather queues


def _balance_blocks(vecs, tgt_chunks):
    """Assign 12544 nodes (rows of vecs: per-class in-edge counts, [12544,4])
    to 98 blocks: block b in half (b // 49), 64 even + 64 odd slots, class
    capacity tgt_chunks[b, cls] * 128 edges. Nodes carry fixed (half,
    parity) from their original local id. Sequential exact-fill: per block,
    greedily pick nodes whose vector tracks the remaining deficit rate so
    per-(block, cls) counts land on the capacity (a multiple of 128).
    Returns slot[l] = final slot index."""
    halves = np.arange(PSLICE) // HALF
    pars = np.arange(PSLICE) & 1
    slot = np.full(PSLICE, -1, np.int64)
    for h in range(2):
        for p in range(2):
            nodes = np.nonzero((halves == h) & (pars == p))[0]
            nv = vecs[nodes]
            # bucket by vector
            keys, inv = np.unique(nv, axis=0, return_inverse=True)
            nk = len(keys)
            members = [[] for _ in range(nk)]
            for i, node in enumerate(nodes):
                members[inv[i]].append(node)
            cnt = np.array([len(m) for m in members], np.int64)
            kf = keys.astype(np.float64)
            for b in range(h * HB, (h + 1) * HB):
                d = (tgt_chunks[b] * 128.0) / 2.0  # per parity share
                picked = 0
                while picked < 64:
                    slots_left = 64 - picked
                    feas = (cnt > 0) & (keys <= d).all(1)
                    if feas.any():
                        rate = d / slots_left
                        score = np.abs(kf - rate).sum(1)
                        score[~feas] = np.inf
                        ki = int(np.argmin(score))
                    else:
                        avail = np.nonzero(cnt > 0)[0]
                        ki = int(avail[np.argmin(kf[avail].sum(1))])
                    node = members[ki].pop()
                    cnt[ki] -= 1
                    d = d - keys[ki]
                    s = picked * 2 + p
                    slot[node] = b * 128 + s
                    picked += 1
    return slot


def _prep(src, dst):
    """Full host-side index preprocessing. Returns per-core arrays and the
    shared run schedule (phase-major: (dst half, src half k), then block,
    then parity)."""
    deg = np.bincount(src, minlength=N).astype(np.float32)

    src_core = src // SLICE
    src_loc = src % SLICE
    dst_core = dst // SLICE
    dst_loc = dst % SLICE
    # classes from ORIGINAL local id (pre-balancing) for capacity planning
    src_half = src_loc // HALF
    src_par = src_loc & 1
    ecls = (src_half * 2 + src_par).astype(np.int64)

    slot_of = np.zeros((CORES, PSLICE), np.int64)
    node_at = np.zeros((CORES, PSLICE), np.int64)
    counts = np.zeros((CORES, NT, CLS), np.int64)
    per_core_edges = []
    for c in range(CORES):
        sel = dst_core == c
        dl = dst_loc[sel]
        vec = np.zeros((PSLICE, CLS), np.int64)
        np.add.at(vec, (dl, ecls[sel]), 1)
        per_core_edges.append((sel, vec))

    # chunk targets per (block, cls), same for all cores. A dst node's half
    # (from its original local id) pins it to that half's 49 blocks, so
    # capacity is budgeted per (dst half, cls).
    tot = np.zeros((CORES, 2, CLS), np.int64)
    for c in range(CORES):
        vec = per_core_edges[c][1]
        tot[c, 0] = vec[:HALF].sum(0)
        tot[c, 1] = vec[HALF:].sum(0)
    need = ((tot.max(0) + 127) // 128) + 2     # [2, CLS] chunks + margin
    tgt = np.zeros((NT, CLS), np.int64)
    for h in range(2):
        for cls in range(CLS):
            q, r = divmod(int(need[h, cls]), HB)
            tgt[h * HB:(h + 1) * HB, cls] = q
            tgt[h * HB:h * HB + r, cls] += 1

    for c in range(CORES):
        sel, vec = per_core_edges[c]
        slot = _balance_blocks(vec, tgt)
        slot_of[c] = slot
        node_at[c, slot] = np.arange(PSLICE)

    # after balancing, recompute real per-(core, block, cls) counts and the
    # final uniform chunk schedule
    for c in range(CORES):
        sel, _ = per_core_edges[c]
        dl = dst_loc[sel]
        b = slot_of[c, dl] >> 7
        np.add.at(counts[c], (b, ecls[sel]), 1)
    cmax = counts.max(0)
    chunks_bc = (cmax + 127) // 128                  # [98, 4]

    # runs: phase-major (src half k OUTER — both k0 phases run before any
    # k1 phase, so the table-half-1 AllGather has a long runway), then dst
    # half h, block asc, parity
    runs = []   # (h, k, b, p, nch)
    for k in range(2):
        for h in range(2):
            for b in range(h * HB, (h + 1) * HB):
                for p in range(2):
                    nch = int(chunks_bc[b, k * 2 + p])
                    if nch:
                        runs.append((h, k, b, p, nch))

    # segments: whole runs of one phase, <= SEG_CHUNKS chunks. Storage
    # within a segment is parity-major ([p0 runs][p1 runs]) so gather
    # calls are parity-pure column slices; compute order stays b-major.
    segments = []   # (h, k, ri0, ri1, seg_cstart, n_p0, nch)
    store = [0] * len(runs)
    cur = 0
    i = 0
    while i < len(runs):
        h, k = runs[i][0], runs[i][1]
        j, tot = i, 0
        while (j < len(runs) and runs[j][0] == h and runs[j][1] == k
               and tot + runs[j][4] <= SEG_CHUNKS):
            tot += runs[j][4]
            j += 1
        np0 = sum(r[4] for r in runs[i:j] if r[3] == 0)
        o0, o1 = cur, cur + np0
        for ri in range(i, j):
            if runs[ri][3] == 0:
                store[ri] = o0
                o0 += runs[ri][4]
            else:
                store[ri] = o1
                o1 += runs[ri][4]
        segments.append((h, k, i, j, cur, np0, tot))
        cur += tot
        i = j
    totch = cur

    # per-core idx / dstlocal arrays
    idx_all, dl_all = [], []
    # src final slot (within its core):
    sslot = slot_of[src_core, src_loc]               # [E]
    s_half = sslot // HALF
    s_par = sslot & 1
    s_pair = (sslot % HALF) >> 1
    # pair row within half table: core*3136 + pair  (checked < 32768)
    s_row = src_core * NPAIR + s_pair
    for c in range(CORES):
        sel, _ = per_core_edges[c]
        dl = dst_loc[sel]
        fs = slot_of[c, dl]
        b = fs >> 7
        dloc = fs & 127
        cls = (s_half[sel] * 2 + s_par[sel]).astype(np.int64)
        key = b * 4 + cls
        order = np.argsort(key, kind="stable")
        keyo = key[order]
        rowo = s_row[sel][order]
        dloco = dloc[order]
        starts = np.searchsorted(keyo, np.arange(NT * 4 + 1))
        idx16 = np.zeros(totch * 128, np.int16)
        dlf = np.full(totch * 128, PAD_DL, np.float32)
        for ri, (h, k, b, p, nch) in enumerate(runs):
            g = b * 4 + k * 2 + p
            s, e = starts[g], starts[g + 1]
            n = e - s
            assert n <= nch * 128, f"overflow {n} > {nch * 128}"
            o = store[ri] * 128
            idx16[o:o + n] = rowo[s:e]
            dlf[o:o + n] = dloco[s:e]
        w = idx16.reshape(totch, 8, 16)
        idx_w = np.tile(np.transpose(w, (2, 0, 1)).reshape(16, totch * 8), (8, 1))
        dl_w = dlf.reshape(totch, 128).T
        idx_all.append(np.ascontiguousarray(idx_w))
        dl_all.append(np.ascontiguousarray(dl_w.astype(ml_dtypes.bfloat16)))

    return deg, node_at, runs, store, segments, totch, idx_all, dl_all


def _build_program(runs, store, segments, totch):
    import concourse.tile as tile
    from concourse import bacc, mybir
    from contextlib import ExitStack

    bf16 = mybir.dt.bfloat16
    f32 = mybir.dt.float32

    nc = bacc.Bacc("TRN2", target_bir_lowering=False, num_swdge_queues=NQ)
    xT_in = nc.dram_tensor("xT", [D + 1, PSLICE], bf16, kind="ExternalInput")
    W1_in = nc.dram_tensor("W1", [D + 1, HID], bf16, kind="ExternalInput")
    W2_in = nc.dram_tensor("W2", [HID, D], bf16, kind="ExternalInput")
    b1_in = nc.dram_tensor("b1w", [P, 2], f32, kind="ExternalInput")
    b2_in = nc.dram_tensor("b2r", [P, D], f32, kind="ExternalInput")
    deg_in = nc.dram_tensor("degw", [P, NT], f32, kind="ExternalInput")
    iota_in = nc.dram_tensor("iota", [P, 2 * P], bf16, kind="ExternalInput")
    idx_in = nc.dram_tensor("idx", [P, totch * 8], mybir.dt.int16,
                            kind="ExternalInput")
    dl_in = nc.dram_tensor("dstloc", [P, totch], bf16, kind="ExternalInput")
    out_ext = nc.dram_tensor("out", [PSLICE, D], f32, kind="ExternalOutput")

    # per (block, k): first / last run index — each (b, k) pair accumulates
    # in its own short-lived PSUM bank (start=True clears has_written for
    # the WHOLE bank, so accumulations must never interleave in a bank)
    first_run = {}
    last_run = {}
    has_k = {}
    for ri, (h, k, b, p, nch) in enumerate(runs):
        if (b, k) not in first_run:
            first_run[(b, k)] = ri
        last_run[(b, k)] = ri
        has_k.setdefault(b, set()).add(k)

    with tile.TileContext(nc) as tc, ExitStack() as ctx:
        const_p = ctx.enter_context(tc.tile_pool(name="const", bufs=1))
        big_p = ctx.enter_context(tc.tile_pool(name="big", bufs=1))
        mlp_p = ctx.enter_context(tc.tile_pool(name="mlp", bufs=2))
        gat_p = ctx.enter_context(tc.tile_pool(name="gat", bufs=5))
        s_p = ctx.enter_context(tc.tile_pool(name="sp", bufs=3))
        meta_p = ctx.enter_context(tc.tile_pool(name="meta", bufs=8))
        fin_p = ctx.enter_context(tc.tile_pool(name="fin", bufs=3))
        mpsum_p = ctx.enter_context(tc.tile_pool(name="mpsum", bufs=1, space="PSUM"))
        ppsum_p = ctx.enter_context(tc.tile_pool(name="ppsum", bufs=1, space="PSUM"))
        dram_p = ctx.enter_context(tc.tile_pool(name="dram", bufs=1, space="DRAM"))

        # ---- constants ----
        W1s = const_p.tile([D + 1, HID], bf16)
        nc.sync.dma_start(W1s[:], W1_in[:, :])
        W2s = const_p.tile([P, 2, D], bf16)
        nc.sync.dma_start(W2s[:], W2_in[:, :].rearrange("(o p) f -> p o f", p=P))
        b1s = const_p.tile([P, 2], f32)
        nc.sync.dma_start(b1s[:], b1_in[:, :])
        b2s = const_p.tile([P, D], f32)
        nc.sync.dma_start(b2s[:], b2_in[:, :])
        iota2_t = const_p.tile([P, P, 2], bf16)
        nc.sync.dma_start(iota2_t[:],
                          iota_in[:, :].rearrange("p (t z) -> p t z", z=2))
        deg_t = const_p.tile([P, NT], f32)
        nc.sync.dma_start(deg_t[:], deg_in[:, :])

        mask_t = const_p.tile([P, NT], f32)
        nc.vector.tensor_scalar(mask_t[:], deg_t[:], 0.0, None,
                                mybir.AluOpType.is_gt)
        degs_t = const_p.tile([P, NT], f32)
        nc.vector.tensor_scalar_max(degs_t[:], deg_t[:], 1.0)
        rec_t = const_p.tile([P, NT], f32)
        nc.vector.reciprocal(rec_t[:], degs_t[:])
        dinv2_t = const_p.tile([P, NT], f32)
        nc.vector.tensor_tensor(dinv2_t[:], rec_t[:], mask_t[:],
                                mybir.AluOpType.mult)
        dsq_t = const_p.tile([P, NT], f32)
        nc.scalar.sqrt(dsq_t[:], rec_t[:])
        # dinv / g0: h_sb holds g0*h, so the table-0 scale is dinv/g0
        dinv10_t = const_p.tile([P, NT], f32)
        nc.vector.tensor_tensor(dinv10_t[:], dsq_t[:], mask_t[:],
                                mybir.AluOpType.mult)
        nc.vector.tensor_scalar_mul(dinv10_t[:], dinv10_t[:], 1.0 / G0)
        dinvg1_t = const_p.tile([P, NT], f32)
        nc.vector.tensor_scalar_mul(dinvg1_t[:], dinv10_t[:], G0 * G1)

        h_sb = big_p.tile([P, NT, D], f32)

        bounce0 = dram_p.tile([PSLICE, D], bf16, tag="b0")
        table0 = dram_p.tile([2 * HROWS, DP], bf16, tag="t0")
        bounce2 = dram_p.tile([PSLICE, D], bf16, tag="b2")
        table2 = dram_p.tile([2 * HROWS, DP], bf16, tag="t2")

        b0pair = bounce0[:].rearrange("(r two) d -> r (two d)", two=2)
        b2pair = bounce2[:].rearrange("(r two) d -> r (two d)", two=2)

        def ag(bpair, table, h):
            nc.gpsimd.collective_compute(
                "AllGather", mybir.AluOpType.bypass,
                replica_groups=[list(range(CORES))],
                ins=[bpair[h * NPAIR:(h + 1) * NPAIR, :].opt()],
                outs=[table[h * HROWS:(h + 1) * HROWS, :].opt()])

        # ---- MLP (block t covers final slots [t*128, (t+1)*128)) ----
        # h_sb holds g0*h (W2/b2 pre-scaled by g0 on host).
        for t in range(NT):
            xt = mlp_p.tile([D + 1, P], bf16, tag="xt")
            nc.sync.dma_start(xt[:], xT_in[:, t * P:(t + 1) * P])
            m1 = mpsum_p.tile([P, 2 * P], f32, tag="m1")
            nc.tensor.matmul(out=m1[:, 0:P], lhsT=W1s[:, 0:P], rhs=xt[:],
                             start=True, stop=True)
            nc.tensor.matmul(out=m1[:, P:2 * P], lhsT=W1s[:, P:HID],
                             rhs=xt[:], start=True, stop=True)
            h1 = mlp_p.tile([P, 2 * P], bf16, tag="h1_s")
            nc.scalar.activation(h1[:], m1[:],
                                 mybir.ActivationFunctionType.Relu,
                                 scale=1.0)
            h1a = h1[:, 0:P]
            h1b = h1[:, P:2 * P]
            m2 = mpsum_p.tile([P, D], f32, tag="m2")
            nc.tensor.matmul(out=m2[:], lhsT=h1a, rhs=W2s[:, 0, :],
                             start=True, stop=False)
            nc.tensor.matmul(out=m2[:], lhsT=h1b, rhs=W2s[:, 1, :],
                             start=False, stop=True)
            nc.vector.tensor_tensor(h_sb[:, t, :], m2[:], b2s[:],
                                    mybir.AluOpType.add)
            t0t = mlp_p.tile([P, D], bf16, tag="t0t")
            nc.scalar.mul(t0t[:], h_sb[:, t, :], dinv10_t[:, t:t + 1])
            nc.sync.dma_start(bounce0[t * P:(t + 1) * P, :], t0t[:])
            if t == HB - 1:
                with tc.high_priority():
                    ag(b0pair, table0, 0)
        with tc.high_priority():
            ag(b0pair, table0, 1)

        def gather128(out_ap, in_ap, idxs_ap, num_idxs, queue_num):
            # dma_gather with 128B elements at 256B stride (parity column
            # slice of the pair table). bass.dma_gather asserts elem_size
            # % 256B == 0 (a transpose-path restriction), so build the
            # instruction directly; the ucode handles arbitrary elem len.
            eng = nc.gpsimd
            eng._assert_queue_num(queue_num)
            _in_ap = eng.lower_ap_dma(in_ap, for_custom_bir_dma=True)
            _idxs_ap = eng.lower_ap(idxs_ap)
            _out_ap = eng.lower_ap(out_ap)
            eng.add_instruction(
                mybir.InstDMAGatherAnt(
                    name=eng.bass.get_next_instruction_name(),
                    ins=[*_in_ap, _idxs_ap,
                         eng.lower_val_access(eng.to_reg(num_idxs))],
                    outs=[_out_ap],
                    transpose=False,
                    num_idxs=num_idxs,
                    elem_size=D,
                    stride_bytes_256=1,
                    gen_mode=0,
                    single_packet=True,
                    queue_num=queue_num,
                    sbuf_tokens_per_rank=0,
                    sbuf_free_dim_per_rank=0,
                    sbuf_free_dim_pad_per_rank=0,
                    sbuf_byte_offset=0,
                ))

        def prop(table, epilogue):
            metas = {}
            ps_of = {}          # (b, k) -> live psum tile
            acc_sb = big_p.tile([P, NT, D], f32, tag="acc")
            done = [False] * NT

            def emit_meta(si):
                if si >= len(segments):
                    return
                _h, _k, ri0, ri1, cstart, np0, nch = segments[si]
                idx_t = meta_p.tile([P, SEG_CHUNKS * 8], mybir.dt.int16,
                                    tag="idx")
                nc.scalar.dma_start(idx_t[:, 0:nch * 8],
                                    idx_in[:, cstart * 8:(cstart + nch) * 8])
                dl_t = meta_p.tile([P, SEG_CHUNKS], bf16, tag="dl")
                nc.scalar.dma_start(dl_t[:, 0:nch],
                                    dl_in[:, cstart:cstart + nch])
                metas[si] = (idx_t, dl_t)

            def emit_load(si):
                _h, k, ri0, ri1, cstart, np0, nch = segments[si]
                idx_t, dl_t = metas.pop(si)
                g_t = gat_p.tile([P, SEG_CHUNKS, D], bf16, tag="g")
                # per parity group: 8-chunk calls (1024 idxs = 64 descs per
                # engine = one 8KB concatenated packet per engine per call)
                qn = si * 16
                for pp, (p0, p1) in enumerate(((0, np0), (np0, nch))):
                    tab_p = table[k * HROWS:(k + 1) * HROWS,
                                  pp * D:(pp + 1) * D]
                    for c0 in range(p0, p1, 8):
                        c1 = min(c0 + 8, p1)
                        gather128(g_t[:, c0:c1, :], tab_p,
                                  idx_t[:, c0 * 8:c1 * 8],
                                  num_idxs=(c1 - c0) * 128,
                                  queue_num=qn % NQ)
                        qn += 1
                nch2 = (nch + 1) // 2
                s_t = s_p.tile([P, SEG_CHUNKS // 2, P, 2], bf16, tag="s")
                dlv = dl_t[:, 0:nch2 * 2].rearrange("p (a z) -> p a z", z=2)
                nc.vector.tensor_tensor(
                    out=s_t[:, 0:nch2, :, :],
                    in0=dlv[:, :, None, :].to_broadcast((P, nch2, P, 2)),
                    in1=iota2_t[:, None, :, :].to_broadcast((P, nch2, P, 2)),
                    op=mybir.AluOpType.is_equal)
                return g_t, s_t

            def emit_compute(si, tiles):
                _h, _k, ri0, ri1, cstart, np0, nch = segments[si]
                g_t, s_t = tiles
                for ri in range(ri0, ri1):
                    rh, rk, b, rp, rn = runs[ri]
                    bs = b
                    bk = (b, rk)
                    if bk not in ps_of:
                        ps_of[bk] = ppsum_p.tile([P, D], f32, tag="ps",
                                                 name="ps", bufs=4)
                    ps = ps_of[bk]
                    final = ri == last_run[bk]
                    st = store[ri] - cstart
                    for jj in range(st, st + rn):
                        nc.tensor.matmul(out=ps[:],
                                         lhsT=s_t[:, jj >> 1, :, jj & 1],
                                         rhs=g_t[:, jj, :],
                                         start=ri == first_run[bk]
                                         and jj == st,
                                         stop=final and jj == st + rn - 1)
                    if not final:
                        continue
                    del ps_of[bk]
                    if rk == 0 and 1 in has_k[b]:
                        # stash k0 partial; k1 phase finishes the block
                        nc.scalar.copy(acc_sb[:, bs, :], ps[:])
                    elif rk == 1 and 0 in has_k[b]:
                        tot = fin_p.tile([P, D], f32, tag="tot")
                        nc.vector.tensor_tensor(tot[:], ps[:],
                                                acc_sb[:, bs, :],
                                                mybir.AluOpType.add)
                        done[b] = True
                        epilogue(b, tot[:])
                    else:
                        done[b] = True
                        epilogue(b, ps[:])

            nseg = len(segments)
            for i in range(min(6, nseg)):
                emit_meta(i)
            tiles = [emit_load(i) for i in range(min(4, nseg))]
            for si in range(nseg):
                emit_meta(si + 6)
                if si + 4 < nseg:
                    tiles.append(emit_load(si + 4))
                emit_compute(si, tiles[si])
                tiles[si] = None
            for b in range(NT):
                if not done[b]:
                    zt = fin_p.tile([P, D], f32, tag="zt")
                    nc.vector.memset(zt[:], 0.0)
                    epilogue(b, zt[:])

        def epi1(b, ps):
            t2t = fin_p.tile([P, D], bf16, tag="t2t")
            nc.scalar.mul(t2t[:], ps, dinv2_t[:, b:b + 1])
            nc.scalar.dma_start(bounce2[b * P:(b + 1) * P, :], t2t[:])
            if b == HB - 1:
                ag(b2pair, table2, 0)
            elif b == NT - 1:
                ag(b2pair, table2, 1)

        prop(table0, epi1)

        def epi2(b, ps):
            tmp = fin_p.tile([P, D], f32, tag="tmp")
            nc.scalar.mul(tmp[:], ps, dinvg1_t[:, b:b + 1])
            o_t = fin_p.tile([P, D], f32, tag="ot")
            nc.vector.tensor_tensor(o_t[:], tmp[:], h_sb[:, b, :],
                                    mybir.AluOpType.add)
            nc.scalar.dma_start(out_ext[b * P:(b + 1) * P, :], o_t[:])

        prop(table2, epi2)

    nc.compile()
    return nc


def kernel(x, edge_index, W1, b1, W2, b2, _trace=False, _tmpdir=None):
    from concourse.bass_utils import run_bass_kernel_spmd

    x = np.asarray(x, dtype=np.float32)
    src = np.asarray(edge_index[0], dtype=np.int64).astype(np.int32)
    dst = np.asarray(edge_index[1], dtype=np.int64).astype(np.int32)
    W1 = np.asarray(W1, dtype=np.float32)
    b1 = np.asarray(b1, dtype=np.float32)
    W2 = np.asarray(W2, dtype=np.float32)
    b2 = np.asarray(b2, dtype=np.float32)

    deg, node_at, runs, store, segments, totch, idx_all, dl_all = _prep(src, dst)

    nc = _build_program(runs, store, segments, totch)

    iota = np.tile(np.repeat(np.arange(P, dtype=np.float32), 2), (P, 1)).astype(
        ml_dtypes.bfloat16)
    b1_w = b1.reshape(2, P).T.astype(np.float32).copy()
    b2_r = np.tile(b2 * G0, (P, 1)).astype(np.float32)
    W1b = np.vstack([W1, b1[None, :]]).astype(ml_dtypes.bfloat16)
    W2b = (W2 * G0).astype(ml_dtypes.bfloat16)
    in_maps = []
    for c in range(CORES):
        # node values in FINAL slot order
        na = node_at[c]                      # slot -> original local id
        xp = np.zeros((PSLICE, D + 1), np.float32)
        xp[:, D] = 1.0
        real = na < SLICE
        xp[real, 0:D] = x[c * SLICE + na[real]]
        xT = np.ascontiguousarray(xp.T).astype(ml_dtypes.bfloat16)
        degc = np.ones(PSLICE, np.float32)
        degc[real] = deg[c * SLICE + na[real]]
        deg_w = np.ascontiguousarray(degc.reshape(NT, P).T)
        in_maps.append({
            "xT": xT, "W1": W1b, "W2": W2b, "b1w": b1_w, "b2r": b2_r,
            "degw": deg_w, "iota": iota,
            "idx": idx_all[c], "dstloc": dl_all[c],
        })

    kw = {}
    if _trace:
        kw.update(trace=True, tmpdir=_tmpdir)
    res = run_bass_kernel_spmd(nc, in_maps, core_ids=list(range(CORES)), **kw)
    out = np.empty((N, D), np.float32)
    for c in range(CORES):
        full = res.results[c]["out"]         # [PSLICE, D] in slot order
        na = node_at[c]
        real = na < SLICE
        out[c * SLICE + na[real]] = full[real]
    if _trace:
        kernel._last_exec_time_ns = res.exec_time_ns
    return out


# revision 20
# speedup vs baseline: 1.1314x; 1.1314x over previous
"""Trainium2 Bass kernel for nn_Event_Encoder (GNN message passing).

out = g0 * h + g1 * prop(prop(h)),  h = MLP(x),
prop(v)[d] = dinv[d] * sum_{e: dst_e = d} dinv[src_e] * v[src_e],
dinv = rsqrt(out-degree), g0 = 0.1, g1 = 0.1 * 0.81.

Strategy (8 NeuronCores, SPMD — one program, per-core data):
  - Nodes sharded 8 x 12544 (12500 real + pad). Each core: MLP on its slice
    (bf16, weights stationary on PE), rows scaled by masked rsqrt(deg).
  - Scaled rows stored compact bf16; PAIRS of consecutive slots form one
    256B dma_gather element, so the gather table is [2 x 25088, 128] bf16,
    half-major: pair row = half*25088 + core*3136 + pair. Two AllGathers
    (one per half) build it; each fires as soon as its half of the producer
    pass is done, and gathers of phase (h, k) only need table half k.
  - Edges partitioned by destination core. Per core, edges bucketed by
    (dst block of 128, src half k, src parity). A host-side balancing
    permutation assigns dst nodes to blocks so the per-(b,k,p) edge counts
    hit near-exact multiples of 128 with one schedule shared by all cores.
  - Schedule is phase-major: (dst half h, src half k), blocks within.
    Each dst block accumulates ALL its chunks (both k phases, both
    parities) into one persistent PSUM slice; a single scalar-engine
    epilogue per block reads PSUM, scales, and DMAs out. 49 live PSUM
    slices per dst half fit in one [128, 49, 64] f32 PSUM tile.
  - Per 128-edge chunk (parity-pure): dma_gather (4 SWDGE queues,
    round-robin) fetches the pair rows, an is_equal against an iota row
    builds the one-hot dst map on DVE, and the TensorEngine accumulates
    one-hot^T @ rows[parity*64:...] into the block's PSUM slice.
  - prop1 epilogue scales by masked 1/deg into bounce2 (bf16); prop2
    epilogue combines with the g0-prescaled fp32 h kept in SBUF. Host
    undoes the permutation.
"""
import numpy as np
import ml_dtypes

N = 100000
D = 64
HID = 256
CORES = 8
SLICE = 12500
PSLICE = 12544          # 98 * 128
NT = 98                 # dst blocks per core
HB = 49                 # dst blocks per half
HALF = 6272             # nodes per half (49 blocks)
NPAIR = HALF // 2       # 3136 pairs per core per half
HROWS = NPAIR * CORES   # 25088 pair rows per half
DP = 128                # pair element: 2 nodes x 64 feats bf16 = 256B
P = 128
SEG_CHUNKS = 64
G0 = 0.1
G1 = 0.1 * (1.0 - 0.1) ** 2
PAD_DL = 200.0
CLS = 4                 # (src half k, src parity p) classes: cls = k*2 + p
NQ = 4                  # SWDGE gather queues


def _balance_blocks(vecs, tgt_chunks):
    """Assign 12544 nodes (rows of vecs: per-class in-edge counts, [12544,4])
    to 98 blocks: block b in half (b // 49), 64 even + 64 odd slots, class
    capacity tgt_chunks[b, cls] * 128 edges. Nodes carry fixed (half,
    parity) from their original local id. Sequential exact-fill: per block,
    greedily pick nodes whose vector tracks the remaining deficit rate so
    per-(block, cls) counts land on the capacity (a multiple of 128).
    Returns slot[l] = final slot index."""
    halves = np.arange(PSLICE) // HALF
    pars = np.arange(PSLICE) & 1
    slot = np.full(PSLICE, -1, np.int64)
    for h in range(2):
        for p in range(2):
            nodes = np.nonzero((halves == h) & (pars == p))[0]
            nv = vecs[nodes]
            # bucket by vector
            keys, inv = np.unique(nv, axis=0, return_inverse=True)
            nk = len(keys)
            members = [[] for _ in range(nk)]
            for i, node in enumerate(nodes):
                members[inv[i]].append(node)
            cnt = np.array([len(m) for m in members], np.int64)
            kf = keys.astype(np.float64)
            for b in range(h * HB, (h + 1) * HB):
                d = (tgt_chunks[b] * 128.0) / 2.0  # per parity share
                picked = 0
                while picked < 64:
                    slots_left = 64 - picked
                    feas = (cnt > 0) & (keys <= d).all(1)
                    if feas.any():
                        rate = d / slots_left
                        score = np.abs(kf - rate).sum(1)
                        score[~feas] = np.inf
                        ki = int(np.argmin(score))
                    else:
                        avail = np.nonzero(cnt > 0)[0]
                        ki = int(avail[np.argmin(kf[avail].sum(1))])
                    node = members[ki].pop()
                    cnt[ki] -= 1
                    d = d - keys[ki]
                    s = picked * 2 + p
                    slot[node] = b * 128 + s
                    picked += 1
    return slot


def _prep(src, dst):
    """Full host-side index preprocessing. Returns per-core arrays and the
    shared run schedule (phase-major: (dst half, src half k), then block,
    then parity)."""
    deg = np.bincount(src, minlength=N).astype(np.float32)

    src_core = src // SLICE
    src_loc = src % SLICE
    dst_core = dst // SLICE
    dst_loc = dst % SLICE
    # classes from ORIGINAL local id (pre-balancing) for capacity planning
    src_half = src_loc // HALF
    src_par = src_loc & 1
    ecls = (src_half * 2 + src_par).astype(np.int64)

    slot_of = np.zeros((CORES, PSLICE), np.int64)
    node_at = np.zeros((CORES, PSLICE), np.int64)
    counts = np.zeros((CORES, NT, CLS), np.int64)
    per_core_edges = []
    for c in range(CORES):
        sel = dst_core == c
        dl = dst_loc[sel]
        vec = np.zeros((PSLICE, CLS), np.int64)
        np.add.at(vec, (dl, ecls[sel]), 1)
        per_core_edges.append((sel, vec))

    # chunk targets per (block, cls), same for all cores. A dst node's half
    # (from its original local id) pins it to that half's 49 blocks, so
    # capacity is budgeted per (dst half, cls).
    tot = np.zeros((CORES, 2, CLS), np.int64)
    for c in range(CORES):
        vec = per_core_edges[c][1]
        tot[c, 0] = vec[:HALF].sum(0)
        tot[c, 1] = vec[HALF:].sum(0)
    need = ((tot.max(0) + 127) // 128) + 2     # [2, CLS] chunks + margin
    tgt = np.zeros((NT, CLS), np.int64)
    for h in range(2):
        for cls in range(CLS):
            q, r = divmod(int(need[h, cls]), HB)
            tgt[h * HB:(h + 1) * HB, cls] = q
            tgt[h * HB:h * HB + r, cls] += 1

    for c in range(CORES):
        sel, vec = per_core_edges[c]
        slot = _balance_blocks(vec, tgt)
        slot_of[c] = slot
        node_at[c, slot] = np.arange(PSLICE)

    # after balancing, recompute real per-(core, block, cls) counts and the
    # final uniform chunk schedule
    for c in range(CORES):
        sel, _ = per_core_edges[c]
        dl = dst_loc[sel]
        b = slot_of[c, dl] >> 7
        np.add.at(counts[c], (b, ecls[sel]), 1)
    cmax = counts.max(0)
    chunks_bc = (cmax + 127) // 128                  # [98, 4]

    # runs: phase-major (src half k OUTER — both k0 phases run before any
    # k1 phase, so the table-half-1 AllGather has a long runway), then dst
    # half h, block asc, parity
    runs = []   # (h, k, b, p, nch)
    for k in range(2):
        for h in range(2):
            for b in range(h * HB, (h + 1) * HB):
                for p in range(2):
                    nch = int(chunks_bc[b, k * 2 + p])
                    if nch:
                        runs.append((h, k, b, p, nch))

    # segments: whole runs of one phase, <= SEG_CHUNKS chunks. Storage
    # within a segment is parity-major ([p0 runs][p1 runs]) so gather
    # calls are parity-pure column slices; compute order stays b-major.
    segments = []   # (h, k, ri0, ri1, seg_cstart, n_p0, nch)
    store = [0] * len(runs)
    cur = 0
    i = 0
    while i < len(runs):
        h, k = runs[i][0], runs[i][1]
        j, tot = i, 0
        while (j < len(runs) and runs[j][0] == h and runs[j][1] == k
               and tot + runs[j][4] <= SEG_CHUNKS):
            tot += runs[j][4]
            j += 1
        np0 = sum(r[4] for r in runs[i:j] if r[3] == 0)
        o0, o1 = cur, cur + np0
        for ri in range(i, j):
            if runs[ri][3] == 0:
                store[ri] = o0
                o0 += runs[ri][4]
            else:
                store[ri] = o1
                o1 += runs[ri][4]
        segments.append((h, k, i, j, cur, np0, tot))
        cur += tot
        i = j
    totch = cur

    # per-core idx / dstlocal arrays
    idx_all, dl_all = [], []
    # src final slot (within its core):
    sslot = slot_of[src_core, src_loc]               # [E]
    s_half = sslot // HALF
    s_par = sslot & 1
    s_pair = (sslot % HALF) >> 1
    # pair row within half table: core*3136 + pair  (checked < 32768)
    s_row = src_core * NPAIR + s_pair
    for c in range(CORES):
        sel, _ = per_core_edges[c]
        dl = dst_loc[sel]
        fs = slot_of[c, dl]
        b = fs >> 7
        dloc = fs & 127
        cls = (s_half[sel] * 2 + s_par[sel]).astype(np.int64)
        key = b * 4 + cls
        order = np.argsort(key, kind="stable")
        keyo = key[order]
        rowo = s_row[sel][order]
        dloco = dloc[order]
        starts = np.searchsorted(keyo, np.arange(NT * 4 + 1))
        idx16 = np.zeros(totch * 128, np.int16)
        dlf = np.full(totch * 128, PAD_DL, np.float32)
        for ri, (h, k, b, p, nch) in enumerate(runs):
            g = b * 4 + k * 2 + p
            s, e = starts[g], starts[g + 1]
            n = e - s
            assert n <= nch * 128, f"overflow {n} > {nch * 128}"
            o = store[ri] * 128
            idx16[o:o + n] = rowo[s:e]
            dlf[o:o + n] = dloco[s:e]
        w = idx16.reshape(totch, 8, 16)
        idx_w = np.tile(np.transpose(w, (2, 0, 1)).reshape(16, totch * 8), (8, 1))
        dl_w = dlf.reshape(totch, 128).T
        idx_all.append(np.ascontiguousarray(idx_w))
        dl_all.append(np.ascontiguousarray(dl_w.astype(ml_dtypes.bfloat16)))

    return deg, node_at, runs, store, segments, totch, idx_all, dl_all


def _build_program(runs, store, segments, totch):
    import concourse.tile as tile
    from concourse import bacc, mybir
    from contextlib import ExitStack

    bf16 = mybir.dt.bfloat16
    f32 = mybir.dt.float32

    nc = bacc.Bacc("TRN2", target_bir_lowering=False, num_swdge_queues=NQ)
    xT_in = nc.dram_tensor("xT", [D + 1, PSLICE], bf16, kind="ExternalInput")
    W1_in = nc.dram_tensor("W1", [D + 1, HID], bf16, kind="ExternalInput")
    W2_in = nc.dram_tensor("W2", [HID, D], bf16, kind="ExternalInput")
    b1_in = nc.dram_tensor("b1w", [P, 2], f32, kind="ExternalInput")
    b2_in = nc.dram_tensor("b2r", [P, D], f32, kind="ExternalInput")
    deg_in = nc.dram_tensor("degw", [P, NT], f32, kind="ExternalInput")
    iota_in = nc.dram_tensor("iota", [P, 2 * P], bf16, kind="ExternalInput")
    idx_in = nc.dram_tensor("idx", [P, totch * 8], mybir.dt.int16,
                            kind="ExternalInput")
    dl_in = nc.dram_tensor("dstloc", [P, totch], bf16, kind="ExternalInput")
    out_ext = nc.dram_tensor("out", [PSLICE, D], f32, kind="ExternalOutput")

    # per (block, k): first / last run index — each (b, k) pair accumulates
    # in its own short-lived PSUM bank (start=True clears has_written for
    # the WHOLE bank, so accumulations must never interleave in a bank)
    first_run = {}
    last_run = {}
    has_k = {}
    for ri, (h, k, b, p, nch) in enumerate(runs):
        if (b, k) not in first_run:
            first_run[(b, k)] = ri
        last_run[(b, k)] = ri
        has_k.setdefault(b, set()).add(k)

    with tile.TileContext(nc) as tc, ExitStack() as ctx:
        const_p = ctx.enter_context(tc.tile_pool(name="const", bufs=1))
        big_p = ctx.enter_context(tc.tile_pool(name="big", bufs=1))
        mlp_p = ctx.enter_context(tc.tile_pool(name="mlp", bufs=2))
        gat_p = ctx.enter_context(tc.tile_pool(name="gat", bufs=5))
        s_p = ctx.enter_context(tc.tile_pool(name="sp", bufs=3))
        meta_p = ctx.enter_context(tc.tile_pool(name="meta", bufs=8))
        fin_p = ctx.enter_context(tc.tile_pool(name="fin", bufs=3))
        mpsum_p = ctx.enter_context(tc.tile_pool(name="mpsum", bufs=1, space="PSUM"))
        ppsum_p = ctx.enter_context(tc.tile_pool(name="ppsum", bufs=1, space="PSUM"))
        dram_p = ctx.enter_context(tc.tile_pool(name="dram", bufs=1, space="DRAM"))

        # ---- constants ----
        W1s = const_p.tile([D + 1, HID], bf16)
        nc.sync.dma_start(W1s[:], W1_in[:, :])
        W2s = const_p.tile([P, 2, D], bf16)
        nc.sync.dma_start(W2s[:], W2_in[:, :].rearrange("(o p) f -> p o f", p=P))
        b1s = const_p.tile([P, 2], f32)
        nc.sync.dma_start(b1s[:], b1_in[:, :])
        b2s = const_p.tile([P, D], f32)
        nc.sync.dma_start(b2s[:], b2_in[:, :])
        iota2_t = const_p.tile([P, P, 2], bf16)
        nc.sync.dma_start(iota2_t[:],
                          iota_in[:, :].rearrange("p (t z) -> p t z", z=2))
        deg_t = const_p.tile([P, NT], f32)
        nc.sync.dma_start(deg_t[:], deg_in[:, :])

        mask_t = const_p.tile([P, NT], f32)
        nc.vector.tensor_scalar(mask_t[:], deg_t[:], 0.0, None,
                                mybir.AluOpType.is_gt)
        degs_t = const_p.tile([P, NT], f32)
        nc.vector.tensor_scalar_max(degs_t[:], deg_t[:], 1.0)
        rec_t = const_p.tile([P, NT], f32)
        nc.vector.reciprocal(rec_t[:], degs_t[:])
        dinv2_t = const_p.tile([P, NT], f32)
        nc.vector.tensor_tensor(dinv2_t[:], rec_t[:], mask_t[:],
                                mybir.AluOpType.mult)
        dsq_t = const_p.tile([P, NT], f32)
        nc.scalar.sqrt(dsq_t[:], rec_t[:])
        # dinv / g0: h_sb holds g0*h, so the table-0 scale is dinv/g0
        dinv10_t = const_p.tile([P, NT], f32)
        nc.vector.tensor_tensor(dinv10_t[:], dsq_t[:], mask_t[:],
                                mybir.AluOpType.mult)
        nc.vector.tensor_scalar_mul(dinv10_t[:], dinv10_t[:], 1.0 / G0)
        dinvg1_t = const_p.tile([P, NT], f32)
        nc.vector.tensor_scalar_mul(dinvg1_t[:], dinv10_t[:], G0 * G1)

        h_sb = big_p.tile([P, NT, D], f32)

        bounce0 = dram_p.tile([PSLICE, D], bf16, tag="b0")
        table0 = dram_p.tile([2 * HROWS, DP], bf16, tag="t0")
        bounce2 = dram_p.tile([PSLICE, D], bf16, tag="b2")
        table2 = dram_p.tile([2 * HROWS, DP], bf16, tag="t2")

        b0pair = bounce0[:].rearrange("(r two) d -> r (two d)", two=2)
        b2pair = bounce2[:].rearrange("(r two) d -> r (two d)", two=2)

        def ag(bpair, table, h):
            nc.gpsimd.collective_compute(
                "AllGather", mybir.AluOpType.bypass,
                replica_groups=[list(range(CORES))],
                ins=[bpair[h * NPAIR:(h + 1) * NPAIR, :].opt()],
                outs=[table[h * HROWS:(h + 1) * HROWS, :].opt()])

        # ---- MLP (block t covers final slots [t*128, (t+1)*128)) ----
        # h_sb holds g0*h (W2/b2 pre-scaled by g0 on host).
        for t in range(NT):
            xt = mlp_p.tile([D + 1, P], bf16, tag="xt")
            nc.sync.dma_start(xt[:], xT_in[:, t * P:(t + 1) * P])
            m1 = mpsum_p.tile([P, 2 * P], f32, tag="m1")
            nc.tensor.matmul(out=m1[:, 0:P], lhsT=W1s[:, 0:P], rhs=xt[:],
                             start=True, stop=True)
            nc.tensor.matmul(out=m1[:, P:2 * P], lhsT=W1s[:, P:HID],
                             rhs=xt[:], start=True, stop=True)
            h1 = mlp_p.tile([P, 2 * P], bf16, tag="h1_s")
            nc.scalar.activation(h1[:], m1[:],
                                 mybir.ActivationFunctionType.Relu,
                                 scale=1.0)
            h1a = h1[:, 0:P]
            h1b = h1[:, P:2 * P]
            m2 = mpsum_p.tile([P, D], f32, tag="m2")
            nc.tensor.matmul(out=m2[:], lhsT=h1a, rhs=W2s[:, 0, :],
                             start=True, stop=False)
            nc.tensor.matmul(out=m2[:], lhsT=h1b, rhs=W2s[:, 1, :],
                             start=False, stop=True)
            nc.vector.tensor_tensor(h_sb[:, t, :], m2[:], b2s[:],
                                    mybir.AluOpType.add)
            t0t = mlp_p.tile([P, D], bf16, tag="t0t")
            nc.scalar.mul(t0t[:], h_sb[:, t, :], dinv10_t[:, t:t + 1])
            nc.sync.dma_start(bounce0[t * P:(t + 1) * P, :], t0t[:])
            if t == HB - 1:
                with tc.high_priority():
                    ag(b0pair, table0, 0)
        with tc.high_priority():
            ag(b0pair, table0, 1)

        def gather128(out_ap, in_ap, idxs_ap, num_idxs, queue_num):
            # dma_gather with 128B elements at 256B stride (parity column
            # slice of the pair table). bass.dma_gather asserts elem_size
            # % 256B == 0 (a transpose-path restriction), so build the
            # instruction directly; the ucode handles arbitrary elem len.
            eng = nc.gpsimd
            eng._assert_queue_num(queue_num)
            _in_ap = eng.lower_ap_dma(in_ap, for_custom_bir_dma=True)
            _idxs_ap = eng.lower_ap(idxs_ap)
            _out_ap = eng.lower_ap(out_ap)
            eng.add_instruction(
                mybir.InstDMAGatherAnt(
                    name=eng.bass.get_next_instruction_name(),
                    ins=[*_in_ap, _idxs_ap,
                         eng.lower_val_access(eng.to_reg(num_idxs))],
                    outs=[_out_ap],
                    transpose=False,
                    num_idxs=num_idxs,
                    elem_size=D,
                    stride_bytes_256=1,
                    gen_mode=0,
                    single_packet=True,
                    queue_num=queue_num,
                    sbuf_tokens_per_rank=0,
                    sbuf_free_dim_per_rank=0,
                    sbuf_free_dim_pad_per_rank=0,
                    sbuf_byte_offset=0,
                ))

        def prop(table, epilogue):
            metas = {}
            ps_of = {}          # (b, k) -> live psum tile
            acc_sb = big_p.tile([P, NT, D], f32, tag="acc")
            done = [False] * NT

            def emit_meta(si):
                if si >= len(segments):
                    return
                _h, _k, ri0, ri1, cstart, np0, nch = segments[si]
                idx_t = meta_p.tile([P, SEG_CHUNKS * 8], mybir.dt.int16,
                                    tag="idx")
                nc.sync.dma_start(idx_t[:, 0:nch * 8],
                                  idx_in[:, cstart * 8:(cstart + nch) * 8])
                dl_t = meta_p.tile([P, SEG_CHUNKS], bf16, tag="dl")
                nc.sync.dma_start(dl_t[:, 0:nch],
                                  dl_in[:, cstart:cstart + nch])
                metas[si] = (idx_t, dl_t)

            def emit_load(si):
                _h, k, ri0, ri1, cstart, np0, nch = segments[si]
                idx_t, dl_t = metas.pop(si)
                g_t = gat_p.tile([P, SEG_CHUNKS, DP], bf16, tag="g")
                # 8-chunk calls: 1024 idxs = 64 descs/engine = one 16KB
                # concatenated packet (single_packet) per engine per call
                for c0 in range(0, nch, 8):
                    c1 = min(c0 + 8, nch)
                    nc.gpsimd.dma_gather(
                        g_t[:, c0:c1, :],
                        table[k * HROWS:(k + 1) * HROWS, :],
                        idx_t[:, c0 * 8:c1 * 8],
                        num_idxs=(c1 - c0) * 128,
                        num_idxs_reg=(c1 - c0) * 128, elem_size=DP,
                        single_packet=True,
                        queue_num=(si * 8 + c0 // 8) % NQ)
                nch2 = (nch + 1) // 2
                s_t = s_p.tile([P, SEG_CHUNKS // 2, P, 2], bf16, tag="s")
                dlv = dl_t[:, 0:nch2 * 2].rearrange("p (a z) -> p a z", z=2)
                nc.vector.tensor_tensor(
                    out=s_t[:, 0:nch2, :, :],
                    in0=dlv[:, :, None, :].to_broadcast((P, nch2, P, 2)),
                    in1=iota2_t[:, None, :, :].to_broadcast((P, nch2, P, 2)),
                    op=mybir.AluOpType.is_equal)
                return g_t, s_t

            def emit_compute(si, tiles):
                _h, _k, ri0, ri1, cstart, np0, nch = segments[si]
                g_t, s_t = tiles
                for ri in range(ri0, ri1):
                    rh, rk, b, rp, rn = runs[ri]
                    bs = b
                    bk = (b, rk)
                    if bk not in ps_of:
                        ps_of[bk] = ppsum_p.tile([P, D], f32, tag="ps",
                                                 name="ps", bufs=4)
                    ps = ps_of[bk]
                    final = ri == last_run[bk]
                    st = store[ri] - cstart
                    off = rp * D
                    for jj in range(st, st + rn):
                        nc.tensor.matmul(out=ps[:],
                                         lhsT=s_t[:, jj >> 1, :, jj & 1],
                                         rhs=g_t[:, jj, off:off + D],
                                         start=ri == first_run[bk]
                                         and jj == st,
                                         stop=final and jj == st + rn - 1)
                    if not final:
                        continue
                    del ps_of[bk]
                    if rk == 0 and 1 in has_k[b]:
                        # stash k0 partial; k1 phase finishes the block
                        nc.scalar.copy(acc_sb[:, bs, :], ps[:])
                    elif rk == 1 and 0 in has_k[b]:
                        tot = fin_p.tile([P, D], f32, tag="tot")
                        nc.vector.tensor_tensor(tot[:], ps[:],
                                                acc_sb[:, bs, :],
                                                mybir.AluOpType.add)
                        done[b] = True
                        epilogue(b, tot[:])
                    else:
                        done[b] = True
                        epilogue(b, ps[:])

            nseg = len(segments)
            for i in range(min(6, nseg)):
                emit_meta(i)
            tiles = [emit_load(i) for i in range(min(4, nseg))]
            for si in range(nseg):
                emit_meta(si + 6)
                if si + 4 < nseg:
                    tiles.append(emit_load(si + 4))
                emit_compute(si, tiles[si])
                tiles[si] = None
            for b in range(NT):
                if not done[b]:
                    zt = fin_p.tile([P, D], f32, tag="zt")
                    nc.vector.memset(zt[:], 0.0)
                    epilogue(b, zt[:])

        def epi1(b, ps):
            t2t = fin_p.tile([P, D], bf16, tag="t2t")
            nc.scalar.mul(t2t[:], ps, dinv2_t[:, b:b + 1])
            nc.scalar.dma_start(bounce2[b * P:(b + 1) * P, :], t2t[:])
            if b == HB - 1:
                ag(b2pair, table2, 0)
            elif b == NT - 1:
                ag(b2pair, table2, 1)

        prop(table0, epi1)

        def epi2(b, ps):
            tmp = fin_p.tile([P, D], f32, tag="tmp")
            nc.scalar.mul(tmp[:], ps, dinvg1_t[:, b:b + 1])
            o_t = fin_p.tile([P, D], f32, tag="ot")
            nc.vector.tensor_tensor(o_t[:], tmp[:], h_sb[:, b, :],
                                    mybir.AluOpType.add)
            nc.scalar.dma_start(out_ext[b * P:(b + 1) * P, :], o_t[:])

        prop(table2, epi2)

    nc.compile()
    return nc


def kernel(x, edge_index, W1, b1, W2, b2, _trace=False, _tmpdir=None):
    from concourse.bass_utils import run_bass_kernel_spmd

    x = np.asarray(x, dtype=np.float32)
    src = np.asarray(edge_index[0], dtype=np.int64).astype(np.int32)
    dst = np.asarray(edge_index[1], dtype=np.int64).astype(np.int32)
    W1 = np.asarray(W1, dtype=np.float32)
    b1 = np.asarray(b1, dtype=np.float32)
    W2 = np.asarray(W2, dtype=np.float32)
    b2 = np.asarray(b2, dtype=np.float32)

    deg, node_at, runs, store, segments, totch, idx_all, dl_all = _prep(src, dst)

    nc = _build_program(runs, store, segments, totch)

    iota = np.tile(np.repeat(np.arange(P, dtype=np.float32), 2), (P, 1)).astype(
        ml_dtypes.bfloat16)
    b1_w = b1.reshape(2, P).T.astype(np.float32).copy()
    b2_r = np.tile(b2 * G0, (P, 1)).astype(np.float32)
    W1b = np.vstack([W1, b1[None, :]]).astype(ml_dtypes.bfloat16)
    W2b = (W2 * G0).astype(ml_dtypes.bfloat16)
    in_maps = []
    for c in range(CORES):
        # node values in FINAL slot order
        na = node_at[c]                      # slot -> original local id
        xp = np.zeros((PSLICE, D + 1), np.float32)
        xp[:, D] = 1.0
        real = na < SLICE
        xp[real, 0:D] = x[c * SLICE + na[real]]
        xT = np.ascontiguousarray(xp.T).astype(ml_dtypes.bfloat16)
        degc = np.ones(PSLICE, np.float32)
        degc[real] = deg[c * SLICE + na[real]]
        deg_w = np.ascontiguousarray(degc.reshape(NT, P).T)
        in_maps.append({
            "xT": xT, "W1": W1b, "W2": W2b, "b1w": b1_w, "b2r": b2_r,
            "degw": deg_w, "iota": iota,
            "idx": idx_all[c], "dstloc": dl_all[c],
        })

    kw = {}
    if _trace:
        kw.update(trace=True, tmpdir=_tmpdir)
    res = run_bass_kernel_spmd(nc, in_maps, core_ids=list(range(CORES)), **kw)
    out = np.empty((N, D), np.float32)
    for c in range(CORES):
        full = res.results[c]["out"]         # [PSLICE, D] in slot order
        na = node_at[c]
        real = na < SLICE
        out[c * SLICE + na[real]] = full[real]
    if _trace:
        kernel._last_exec_time_ns = res.exec_time_ns
    return out
